# revision 62
# baseline (speedup 1.0000x reference)
"""Trainium2 Bass kernel for nn_AlternateAttention (3-block transformer:
global attention, lower-band attention, upper-band attention, each with MLP).

Sharding: 8 cores = 2 batches x 4 sequence chunks of 1024 tokens, each core
processing an extended window of 1152 tokens (64-token halo each side) so the
banded blocks need no inter-core communication. Block 0 (global attention)
needs full-sequence K/V; each core recomputes them from the replicated raw
input (LN1+KV projection over the full 4096 tokens of its batch).

Layout: activations live TRANSPOSED in SBUF ([feature, token]); residual
stream in bf16. Attention projections (qkv/out) and the MLP's fc1 run in
fp8e4 DoubleRow (2 K-slabs per instruction at 0.5 cycles/col); fc2 stays
bf16 (a second fp8 MLP stage pushes rel-err past the 2e-2 gate). Scores run
fp8 K^T (stationary) against fp8 Q (moving); probs come out of exp in fp8
and feed a DoubleRow AV over k-chunk pairs. Scales: h x8, W x2048, Q/K/V
x64. Softmax normalizer rides as a ones-row in V.

Block-0 LN1 is mean-FOLDED: stats are taken on host-provided AX-scaled fp8
x8 and W@LN(x) is computed as rstd*(W@x8 - mean*colsum(W8)) - the rank-1
mean term is a 1-row matmul into the same psum, rstd (carrying the output
scale) multiplies at the store (row-broadcast for Q/K^T, per-partition
column via PE-transpose for V). The normalized h never materializes.

Engines: exp is batched 2 PSUM banks per instruction and the table-using
ACT instructions (exp/ln vs gelu) are totally ordered in emission order via
add_dep_helper so the scheduler cannot thrash the activation-table RAM.
Block-0 attention is software-pipelined over (qtile, head) units: scores+exp
of unit N overlap the AV chain of unit N-1, and each qtile's out-proj/LN2/
MLP tail is deferred one extra unit so the PE MLP stretch hides behind two
heads of exp backlog. Banded-block K/Q/V stores run on ACT (scalar-scale
copies; that region is DVE-bound), block-0 KV stores on DVE/ACT split.
"""
import contextlib
import numpy as np
import ml_dtypes

# ---- problem constants (hardcoded per contract) ----
B, S, E, H, W_BAND, FF = 2, 4096, 512, 8, 16, 2048
HD = E // H                      # 64
N_CORES = 8
CHUNKS = 4                       # sequence chunks per batch
OWN = S // CHUNKS                # 1024
HALO = 64
T = OWN + 2 * HALO               # 1152 = 9*128
P = 128
NT = 384                         # token tile (3 per T)
NQT = T // NT                    # 3
NKC = S // P                     # 32 k-chunks for global attention
NTC = T // P                     # 9 token chunks of ext window
EC = E // P                      # 4 feature strips
FFC = FF // P                    # 16

# fp8 scale plan
ALPHA = 8.0                      # LN1 output scale (fp8 h)
BETA = 2048.0                    # qkv/out weight scale (w std 0.02)
GQ = 64.0                        # Q scale in fp8
GK = 64.0                        # K scale in fp8
GV = 64.0                        # V scale in fp8

EXT_STARTS = [max(0, min(OWN * c - HALO, S - T)) for c in range(CHUNKS)]
OWN_OFF = [OWN * c - EXT_STARTS[c] for c in range(CHUNKS)]

_EXEC_CACHE = {}
_PHASE_MARKS = []
_PHASE_OF = {}

bf16 = ml_dtypes.bfloat16
e4m3 = ml_dtypes.float8_e4m3


# ------------------------------------------------------------------
# device program
# ------------------------------------------------------------------
def _patch_act_tables():
    # The bacc table-load placement maps exp -> 'exp_and_others' and
    # ln -> 'natural_log', thrashing the ACT table RAM (~1.3us per switch,
    # dozens of switches). Restrict the choice to the two sets that cover
    # everything this kernel uses so exp/ln never evict each other.
    import concourse.hw_specs as hw_specs
    import concourse.bacc as bacc_mod
    import concourse.bass_interp as bass_interp
    if getattr(hw_specs, "_aa_patched", False):
        return
    orig = hw_specs.get_activation_tables
    keep = {"natural_log_exp_and_others", "gelu_apprx_tanh_and_others"}

    def _gat(arch):
        tabs = orig(arch)
        return {k: (v if k in keep else set()) for k, v in tabs.items()}

    hw_specs.get_activation_tables = _gat
    bacc_mod.get_activation_tables = _gat
    bass_interp.get_activation_tables = _gat
    hw_specs._aa_patched = True


def _build_nc(use_bias: bool, repeat: int = 1):
    import concourse.bacc as bacc
    import concourse.mybir as mybir
    import concourse.tile as tile

    _patch_act_tables()

    f32 = mybir.dt.float32
    b16 = mybir.dt.bfloat16
    f8 = mybir.dt.float8e4
    AF = mybir.ActivationFunctionType
    OP = mybir.AluOpType
    PM = mybir.MatmulPerfMode

    nc = bacc.Bacc("TRN2", target_bir_lowering=False, debug=False,
                   num_devices=N_CORES)
    _PHASE_MARKS.clear()

    def mark(label):
        n = sum(len(b.instructions) for b in nc.main_func.blocks)
        _PHASE_MARKS.append((label, n))

    # ---- dram tensors ----
    xT = nc.dram_tensor("xT", [E, T], b16, kind="ExternalInput")
    # block-0 LN1 inputs: AX-scaled fp8 copies of x (ext window + full seq)
    # plus the qkv0 weight column sums for the folded-mean rank-1 correction
    xT8 = nc.dram_tensor("xT8", [E, T], f8, kind="ExternalInput")
    xfT8 = nc.dram_tensor("xfT8", [E, S], f8, kind="ExternalInput")
    csum0 = nc.dram_tensor("csum0", [1, 3 * E], b16, kind="ExternalInput")
    wq, wo, w1, w2, bias_d = [], [], [], [], []
    for i in range(3):
        wq.append(nc.dram_tensor(f"qkvT8_{i}", [EC, P, 3 * E], f8, kind="ExternalInput"))
        wo.append(nc.dram_tensor(f"outT8_{i}", [EC, P, E], f8, kind="ExternalInput"))
        w1.append(nc.dram_tensor(f"fc1T{i}", [EC, P, FF], f8, kind="ExternalInput"))
        w2.append(nc.dram_tensor(f"fc2T{i}", [FFC, P, E], b16, kind="ExternalInput"))
        if use_bias:
            # packed per-feature biases for transposed-layout outputs,
            # pre-scaled host-side to match each consumer's psum scale:
            # [q(4xP) | k(4xP) | out(4xP) | fc1(16xP) | fc2(4xP)] -> [P, 32]
            bias_d.append(nc.dram_tensor(f"bias{i}", [P, 32], f32, kind="ExternalInput"))
            bias_d.append(nc.dram_tensor(f"vbias{i}", [1, E], b16, kind="ExternalInput"))
    # 0/1 bf16 packed band masks: each head's <=4 (kc, window) segments
    # concatenated along columns (lo3/lo4/up3/up4 variants)
    mask_d = {}
    for nm in ("mp_lo3", "mp_lo4", "mp_up3", "mp_up4"):
        mask_d[nm] = nc.dram_tensor(nm, [P, 448], b16, kind="ExternalInput")
    yT = nc.dram_tensor("yT", [E, T], b16, kind="ExternalOutput")

    from concourse.tile import add_dep_helper as _adh

    with tile.TileContext(nc) as tc, contextlib.ExitStack() as ctx:
        pool = lambda name, bufs, **kw: ctx.enter_context(
            tc.tile_pool(name=name, bufs=bufs, **kw))

        # Total-order the table-using ACT instructions (exp/ln vs gelu) in
        # emission order: the tile scheduler otherwise interleaves the two
        # sets freely and the activation-table RAM reloads (~1.3us) dozens
        # of times. Copy/identity live in every set and float freely.
        _tbl_funcs = {AF.Exp, AF.Ln, AF.Gelu_apprx_tanh}
        _last_tbl = [None]

        def act(out, in_, func, **kw):
            r = nc.scalar.activation(out, in_, func, **kw)
            if func in _tbl_funcs:
                ins = getattr(r, "ins", r)
                if _last_tbl[0] is not None:
                    _adh(ins, _last_tbl[0], sync=False,
                         reason="act-table emission order")
                _last_tbl[0] = ins
            return r

        # ---- pools live for the whole kernel ----
        p_x = pool("x", 2)           # residual strips bf16, tags x0..x3
        p_h = pool("h", 1)           # LN output tile [P, EC, Tn]
        p_sq = pool("sq", 2)         # per-chunk squares bf16
        p_qt = pool("qt", 1)         # QT [P, EC, T] fp8
        p_ot = pool("ot", 1)         # attention out [P, EC, T] fp8
        p_wqkv = pool("wqkv", 1)     # qkv weights [P, EC, 3E] fp8
        p_wout = pool("wout", 1)
        p_tmp = pool("tmp", 2)       # temporaries
        p_sm = pool("sm", 2)         # small [1, n] stat vectors
        p_c = pool("const", 1)       # ones, masks, biases
        p_g = pool("g", 2 if not use_bias else 1)    # gelu out bf16
        p_wfc1 = pool("wfc1", 1)
        p_wfc2 = pool("wfc2", 1)
        # PSUM: exactly 8 banks, hand-assigned tags
        p_ps = pool("ps", 1, space="PSUM")
        # sc01/sc23: two 2-bank score tiles [P, 2, 512] - scores land in
        #   halves, ONE batched exp reads both banks (halves ACT overhead);
        #   fc1 reuses them for batched gelu pairs
        # po0/po1: attention output accumulators (+ LN stat sums s1/s2,
        #   temporally disjoint)
        # mm0/mm1: generic gemm accumulators

        ones = p_c.tile([P, P], b16)
        nc.vector.memset(ones[:], 1.0)
        ones8 = p_c.tile([P, 1], f8, tag="ones8", name="ones8")
        nc.vector.memset(ones8[:], 1.0)
        eps1 = p_c.tile([1, 1], f32)
        nc.vector.memset(eps1[:], 1e-5)
        lnA = p_c.tile([1, 1], f32)
        nc.vector.memset(lnA[:], float(np.log(ALPHA)))
        # block-0 folded-LN constants: eps in AX^2 scale, and the combined
        # store factor ln(G/BETA) baked into the rstd exp
        eps8 = p_c.tile([1, 1], f32, tag="eps8", name="eps8")
        nc.vector.memset(eps8[:], float(ALPHA * ALPHA * 1e-5))
        lnGB = p_c.tile([1, 1], f32, tag="lnGB", name="lnGB")
        nc.vector.memset(lnGB[:], float(np.log(GK / BETA)))
        c8 = p_c.tile([1, 3 * E], b16, tag="c8", name="c8")
        nc.sync.dma_start(c8[:], csum0[:])
        masks = {}
        for nm, d in mask_d.items():
            mt = p_c.tile([P, 448], b16, tag=f"mask_{nm}", name=f"mask_{nm}")
            nc.sync.dma_start(mt[:], d[:])
            masks[nm] = mt
        bias_t, vbias_t = [], []
        if use_bias:
            for i in range(3):
                bt = p_c.tile([P, 32], f32, tag=f"bias{i}", name=f"bias{i}")
                nc.sync.dma_start(bt[:], bias_d[2 * i][:])
                bias_t.append(bt)
                vt = p_c.tile([1, E], b16, tag=f"vbias{i}", name=f"vbias{i}")
                nc.sync.dma_start(vt[:], bias_d[2 * i + 1][:])
                vbias_t.append(vt)

        def bslice(i, group, oc):
            base = {"qkv": 0, "out": 8, "fc1": 12, "fc2": 28}[group]
            return bias_t[i][:, base + oc:base + oc + 1]

        def add_vbias(i, ps):
            # V projection output is in normal layout [token, feat]: bias
            # varies along the free dim -> broadcast with a K=1 matmul.
            vb = p_ps.tile([P, 512], f32, tag="po0", name="vb")
            nc.tensor.matmul(vb[:], ones[0:1, :], vbias_t[i][:],
                             start=True, stop=True)
            vbs = p_tmp.tile([P, 512], f32, tag="vbs", name="vbs")
            nc.vector.tensor_copy(vbs[:], vb[:])
            nc.vector.tensor_add(ps[:], ps[:], vbs[:])

        def psum_scaled(dst_ap, ps_ap, scale, i, group, oc):
            """dst = ps*scale (+bias) on DVE; bias pre-scaled host-side"""
            b = bslice(i, group, oc) if use_bias else None
            with nc.allow_low_precision(reason="fp8/bf16 activation store"):
                if b is None:
                    nc.vector.tensor_scalar(dst_ap, ps_ap, scale, None, OP.mult)
                else:
                    nc.vector.tensor_scalar(dst_ap, ps_ap, scale, b,
                                            OP.mult, OP.add)

        def _emit_once():
            # fp8 x for block-0 LN1/Q first: it gates the very first compute,
            # while the residual strips aren't read until out-proj
            x8e = p_h.tile([P, EC, T], f8, tag="h", name="x8e")
            for s in range(EC):
                nc.sync.dma_start(x8e[:, s, :], xT8[P * s:P * (s + 1), :])
            xs = []
            for s in range(EC):
                t = p_x.tile([P, T], b16, tag=f"x{s}", name=f"xin{s}")
                nc.sync.dma_start(t[:], xT[P * s:P * (s + 1), :])
                xs.append(t)

            # ---------- layernorm ----------
            def ln_chunk(x_strips, ht, o, n, out_f8, sq_on_pool=False):
                    sl = slice(o, o + n)
                    s1 = p_ps.tile([1, 512], f32, tag="po0", name="s1")
                    s2 = p_ps.tile([1, 512], f32, tag="po1", name="s2")
                    for s in range(EC):
                        rhs_x = x_strips[s][:, sl]
                        nc.tensor.matmul(s1[:, :n], ones[:, 0:1], rhs_x,
                                         start=(s == 0), stop=(s == EC - 1))
                        sq_nt = p_sq.tile([P, 512], b16, tag="sqn", name="sqn")
                        nc.vector.tensor_mul(sq_nt[:, :n], rhs_x, rhs_x)
                        nc.tensor.matmul(s2[:, :n], ones[:, 0:1], sq_nt[:, :n],
                                         start=(s == 0), stop=(s == EC - 1))
                    m_b = p_sm.tile([1, 512], b16, tag="m_b", name="m_b")
                    nc.vector.tensor_scalar(m_b[:, :n], s1[:, :n], 1.0 / E, None, OP.mult)
                    stt = p_sm.tile([1, 1024], f32, tag="stt", name="stt")
                    sa, sb = stt[:, 0:n], stt[:, 512:512 + n]
                    nc.vector.tensor_scalar(sa, s2[:, :n], 1.0 / E, None, OP.mult)
                    nc.vector.tensor_mul(sb, m_b[:, :n], m_b[:, :n])
                    nc.vector.tensor_sub(sa, sa, sb)
                    act(sb, sa, AF.Ln, bias=eps1[:])
                    lnv = sb
                    r_b = p_sm.tile([1, 512], b16, tag="r_b", name="r_b")
                    # rstd (scaled by ALPHA for fp8 h): exp(-ln(v)/2 + lnA)
                    act(r_b[:, :n], lnv, AF.Exp, scale=-0.5,
                        bias=lnA[:] if out_f8 else 0.0)
                    # broadcast per-token mean/rstd on the (idle) Pool engine
                    mBs = p_tmp.tile([P, 512], b16, tag="mBs", name="mBs")
                    nc.gpsimd.partition_broadcast(mBs[:, :n], m_b[:, :n])
                    rBs = p_tmp.tile([P, 512], b16, tag="rBs", name="rBs")
                    nc.gpsimd.partition_broadcast(rBs[:, :n], r_b[:, :n])
                    for s in range(EC):
                        t0 = p_tmp.tile([P, 512], b16, tag="lnt", name="lnt")
                        nc.vector.tensor_sub(t0[:, :n], x_strips[s][:, sl],
                                             mBs[:, :n])
                        with nc.allow_low_precision(reason="fp8 h store"):
                            nc.vector.tensor_mul(ht[:, s, sl], t0[:, :n],
                                                 rBs[:, :n])

            def layernorm(x_strips, Tn, htag, out_f8):
                """x_strips: 4 [P, Tn] bf16 -> one [P, EC, Tn] tile
                (fp8 scaled by ALPHA, or bf16 unscaled)"""
                ht = p_h.tile([P, EC, Tn], f8 if out_f8 else b16,
                              tag=htag, name=htag)
                step = 512 if Tn % 512 == 0 else NT
                for k in range((Tn + step - 1) // step):
                    o = k * step
                    ln_chunk(x_strips, ht, o, min(step, Tn - o), out_f8)
                return ht

            # ---------- folded LN stats on AX-scaled fp8 x (block 0) ----------
            # The normalized h never materializes: W@h = r*(W@x8 - m8*colsum(W))
            # with stats taken on x8 = AX*x. The rank-1 mean term is a 1-row
            # matmul into the same psum; rstd (carrying the G/BETA output
            # scale) multiplies at the store.
            def ln_stats8(x8, o, n):
                """returns (nm, r_b): [1,n] bf16 rows, nm = -mean(x8),
                r_b = (G/BETA)*exp(-0.5*ln(var(x8)+AX^2*eps))"""
                sl = slice(o, o + n)
                s1 = p_ps.tile([1, 512], f32, tag="po0", name="s1")
                s2 = p_ps.tile([1, 512], f32, tag="po1", name="s2")
                for s in range(EC):
                    rhs_x = x8[:, s, sl]
                    nc.tensor.matmul(s1[:, :n], ones8[:, 0:1], rhs_x,
                                     start=(s == 0), stop=(s == EC - 1))
                    sq_nt = p_sq.tile([P, 512], b16, tag="sqn", name="sqn")
                    # fp8 inputs get no DVE 2x; split squares DVE/ACT
                    if s % 2 == 0:
                        nc.vector.tensor_mul(sq_nt[:, :n], rhs_x, rhs_x)
                    else:
                        act(sq_nt[:, :n], rhs_x, AF.Square)
                    nc.tensor.matmul(s2[:, :n], ones[:, 0:1], sq_nt[:, :n],
                                     start=(s == 0), stop=(s == EC - 1))
                nm = p_sm.tile([1, 512], b16, tag="m_b", name="nm")
                nc.vector.tensor_scalar(nm[:, :n], s1[:, :n], -1.0 / E, None,
                                        OP.mult)
                stt = p_sm.tile([1, 1024], f32, tag="stt", name="stt")
                sa, sb = stt[:, 0:n], stt[:, 512:512 + n]
                nc.vector.tensor_scalar(sa, s2[:, :n], 1.0 / E, None, OP.mult)
                nc.vector.tensor_mul(sb, nm[:, :n], nm[:, :n])
                nc.vector.tensor_sub(sa, sa, sb)
                act(sb, sa, AF.Ln, bias=eps8[:])
                r_b = p_sm.tile([1, 512], b16, tag="r_b", name="r_b")
                act(r_b[:, :n], sb, AF.Exp, scale=-0.5, bias=lnGB[:])
                return nm, r_b

            def fold_proj(ps, wcol0, x8, o, n, nm):
                """psum = W8[:, wcol0:wcol0+P] @ x8[:, o:o+n] - m8*colsum"""
                for j in range(EC // 2):
                    nc.tensor.matmul(ps[:, :n],
                                     wqkv0[:, 2 * j:2 * j + 2,
                                           wcol0:wcol0 + P],
                                     x8[:, 2 * j:2 * j + 2, o:o + n],
                                     start=(j == 0), stop=False,
                                     perf_mode=PM.DoubleRow)
                nc.tensor.matmul(ps[:, :n], c8[0:1, wcol0:wcol0 + P],
                                 nm[:, :n], start=False, stop=True,
                                 skip_group_check=True)

            # ---------- transposed GEMM ----------
            _gm_cycle = [0]
            _sc_cycle = [0]
            _po_cycle = [0]

            def ps_tile(cycle=False):
                t = ("mm0", "mm1")[_gm_cycle[0] % 2]
                _gm_cycle[0] += 1
                return p_ps.tile([P, 512], f32, tag=t, name="gps")

            def sc_tile():
                """2-bank score tile [P, 2, 512]"""
                t = ("sc01", "sc23")[_sc_cycle[0] % 2]
                _sc_cycle[0] += 1
                return p_ps.tile([P, 2, 512], f32, tag=t, name="sps")

            def po_tile(tags=("po0", "po1")):
                t = tags[_po_cycle[0] % len(tags)]
                _po_cycle[0] += 1
                return p_ps.tile([HD + 1, 512], f32, tag=t, name="po")

            def gemm8(w_tile, col0, n_oc, h_tile, Tn, post, chunks=None):
                """fp8 DoubleRow: out^T[oc] summed over K-slab pairs"""
                step = 512 if Tn % 512 == 0 else NT
                nss = chunks or [(k * step, min(step, Tn - k * step))
                                 for k in range((Tn + step - 1) // step)]
                for (o, n) in nss:
                    for oc in range(n_oc):
                        ps = ps_tile()
                        for j in range(EC // 2):
                            nc.tensor.matmul(
                                ps[:, :n],
                                w_tile[:, 2 * j:2 * j + 2,
                                       col0 + oc * P:col0 + (oc + 1) * P],
                                h_tile[:, 2 * j:2 * j + 2, o:o + n],
                                start=(j == 0), stop=(j == EC // 2 - 1),
                                perf_mode=PM.DoubleRow)
                        post(oc, o, n, ps)

            # ---------- attention core (shared) ----------
            _fin_cycle = [0]

            def _finish_attn(po, qt0, qn, hp, hh, ot_strips, banded=False):
                ou = p_tmp.tile([HD + 1, 512], b16, tag="ou", name="ou")
                with nc.allow_low_precision(reason="bf16 softmax normalizer"):
                    if banded:
                        act(ou[:, :qn], po[:, :qn], AF.Copy)  # frees po bank
                    else:
                        nc.vector.tensor_copy(ou[:, :qn], po[:, :qn])
                    linv = p_sm.tile([1, 512], b16, tag="linv", name="linv")
                    nc.vector.reciprocal(linv[:, :qn], ou[HD:HD + 1, :qn])
                # 1/l broadcast AND final scale both on Pool: the DVE
                # in-order stream never waits on a Pool round-trip
                lBs = p_tmp.tile([P, 512], b16, tag="lBs", name="lBs")
                nc.gpsimd.partition_broadcast(lBs[:HD, :qn], linv[:, :qn])
                _fin_cycle[0] += 1
                if banded and _fin_cycle[0] % 2 == 0:
                    with nc.allow_low_precision(reason="fp8 attn out"):
                        nc.vector.tensor_mul(
                            ot_strips[hp][HD * hh:HD * (hh + 1), qt0:qt0 + qn],
                            ou[:HD, :qn], lBs[:HD, :qn])
                else:
                    nc.gpsimd.tensor_mul(
                        ot_strips[hp][HD * hh:HD * (hh + 1), qt0:qt0 + qn],
                        ou[:HD, :qn], lBs[:HD, :qn])

            def attn_scores_exps(kt_tile, q_tile, h_, qt0, qn):
                """global attention scores+exp for one (head, qtile): emits
                score matmuls ping-ponged through the 2-bank sc tiles with
                one batched exp per group; returns the fp8 prob tiles"""
                hp, hh = h_ // 2, h_ % 2
                slots = max(1, min(4, 512 // qn))
                grp = 2 * slots
                prs = []   # (kc, pr, slot)
                for g0 in range(0, NKC, grp):
                    gk = list(range(g0, min(g0 + grp, NKC)))
                    sps = sc_tile()
                    for m, kc in enumerate(gk):
                        nc.tensor.matmul(
                            sps[:, m % 2, (m // 2) * qn:(m // 2 + 1) * qn],
                            kt_tile[HD * hh:HD * (hh + 1), hp,
                                    kc * P:(kc + 1) * P],
                            q_tile[HD * hh:HD * (hh + 1), hp,
                                   qt0:qt0 + qn],
                            start=True, stop=True)
                    ncol = ((len(gk) + 1) // 2) * qn
                    pr = p_probs.tile([P, 2, 512], f8, tag="pr", name="pr")
                    act(pr[:, :, :ncol], sps[:, :, :ncol], AF.Exp,
                        scale=0.125 / (GQ * GK))
                    for c in range(len(gk) // 2):
                        prs.append((gk[2 * c], pr, c))
                return prs

            def attn_avs(v_tile, ot_strips, h_, qt0, qn, prs):
                """DoubleRow AV chain + finish for a (head, qtile) whose
                probs were emitted earlier (software-pipelined vs scores)"""
                hp, hh = h_ // 2, h_ % 2
                po = po_tile()
                for idx, (kc, pr, c) in enumerate(prs):
                    nc.tensor.matmul(po[:, :qn],
                                     v_tile[:, kc:kc + 2, h_, 0:HD + 1],
                                     pr[:, :, c * qn:c * qn + qn],
                                     start=(idx == 0),
                                     stop=(idx == len(prs) - 1),
                                     perf_mode=PM.DoubleRow)
                _finish_attn(po, qt0, qn, hp, hh, ot_strips)

            # ---------- MLP building blocks (shared; bf16) ----------
            def mlp_weights(i):
                wf1 = p_wfc1.tile([P, EC, FF], f8, tag="wfc1", name=f"wfc1_{i}")
                for ec in range(EC):
                    nc.sync.dma_start(wf1[:, ec, :], w1[i][ec])
                wf2 = p_wfc2.tile([P, FFC, E], b16, tag="wfc2", name=f"wfc2_{i}")
                for fc in range(FFC):
                    nc.sync.dma_start(wf2[:, fc, :], w2[i][fc])
                return wf1, wf2

            def mlp_nt(i, x_strips, x_new, h2, wf1, wf2, nt, use_mm=False):
                # fc1 in fp8 DoubleRow: h2 is ALPHA-scaled fp8, w1 BETA-scaled
                # fp8; psum1 = ALPHA*BETA*fc1out, gelu applies 1/(ALPHA*BETA).
                # g/fc2 stay bf16: a second fp8 stage pushes rel-err past 2e-2.
                o0 = nt * NT
                g = p_g.tile([P, FFC, NT], b16, tag="g", name="g")
                if use_mm:
                    # block 0: attention owns the sc tiles; fc1 goes through
                    # the mm banks with per-fc gelu
                    for fc in range(FFC):
                        ps = ps_tile()
                        for j in range(EC // 2):
                            nc.tensor.matmul(ps[:, :NT],
                                             wf1[:, 2 * j:2 * j + 2,
                                                 fc * P:(fc + 1) * P],
                                             h2[:, 2 * j:2 * j + 2, o0:o0 + NT],
                                             start=(j == 0), stop=(j == EC // 2 - 1),
                                             perf_mode=PM.DoubleRow)
                        if use_bias:
                            nc.vector.tensor_scalar(ps[:, :NT], ps[:, :NT],
                                                    bslice(i, "fc1", fc), None, OP.add)
                        act(g[:, fc, :], ps[:, :NT], AF.Gelu_apprx_tanh,
                            scale=1.0 / (ALPHA * BETA))
                else:
                    for fcp in range(FFC // 2):
                        # fc1 pairs into the two banks of a score tile
                        # (idle during MLP), one batched gelu per pair
                        sps = sc_tile()
                        for j in range(2):
                            fc = 2 * fcp + j
                            for k in range(EC // 2):
                                nc.tensor.matmul(sps[:, j, :NT],
                                                 wf1[:, 2 * k:2 * k + 2,
                                                     fc * P:(fc + 1) * P],
                                                 h2[:, 2 * k:2 * k + 2, o0:o0 + NT],
                                                 start=(k == 0), stop=(k == EC // 2 - 1),
                                                 perf_mode=PM.DoubleRow)
                            if use_bias:
                                nc.vector.tensor_scalar(sps[:, j, :NT], sps[:, j, :NT],
                                                        bslice(i, "fc1", fc), None, OP.add)
                        act(g[:, 2 * fcp:2 * fcp + 2, :], sps[:, :, :NT],
                            AF.Gelu_apprx_tanh, scale=1.0 / (ALPHA * BETA))
                for oc in range(EC):
                    ps = ps_tile()
                    for fc in range(FFC):
                        nc.tensor.matmul(ps[:, :NT],
                                         wf2[:, fc, oc * P:(oc + 1) * P],
                                         g[:, fc, :],
                                         start=(fc == 0), stop=(fc == FFC - 1))
                    if use_bias:
                        nc.vector.tensor_scalar(ps[:, :NT], ps[:, :NT],
                                                bslice(i, "fc2", oc), None, OP.add)
                    with nc.allow_low_precision(reason="bf16 residual"):
                        nc.vector.tensor_add(x_new[oc][:, o0:o0 + NT],
                                             x_strips[oc][:, o0:o0 + NT],
                                             ps[:, :NT])

            # ==================================================================
            # BLOCK 0: global attention
            # ==================================================================
            wqkv0 = p_wqkv.tile([P, EC, 3 * E], f8, tag="wqkv", name="wqkv0")
            for ec in range(EC):
                nc.sync.dma_start(wqkv0[:, ec, :], wq[0][ec])
            wout = p_wout.tile([P, EC, E], f8, tag="wout", name="wout0")
            for ec in range(EC):
                nc.sync.dma_start(wout[:, ec, :], wo[0][ec])

            mark("b0.ln1+q")
            # folded-LN Q projection straight from fp8 x8 (no h tile at all)
            qt_t = p_qt.tile([P, EC, T], f8, tag="qt", name="qt0")
            for k in range(NQT):
                o = k * NT
                nm, r_b = ln_stats8(x8e, o, NT)
                rBs = p_tmp.tile([P, 512], b16, tag="rBs", name="rBs")
                nc.gpsimd.partition_broadcast(rBs[:, :NT], r_b[:, :NT])
                for oc in range(EC):
                    ps = ps_tile()
                    fold_proj(ps, oc * P, x8e, o, NT, nm)
                    with nc.allow_low_precision(reason="fp8 Q store"):
                        nc.vector.tensor_mul(qt_t[:, oc, o:o + NT],
                                             ps[:, :NT], rBs[:, :NT])
                        if use_bias:
                            nc.vector.tensor_scalar(
                                qt_t[:, oc, o:o + NT], qt_t[:, oc, o:o + NT],
                                bslice(0, "qkv", oc), None, OP.add)

            mark("b0.kv")
            ot0 = p_ot.tile([P, EC, T], f8, tag="ot", name="ot0")
            ot_strips = [ot0[:, s, :] for s in range(EC)]
            with tc.tile_pool(name="kvfull", bufs=1) as p_kv, \
                 tc.tile_pool(name="xpan", bufs=4 if not use_bias else 1) as p_xp, \
                 tc.tile_pool(name="probs", bufs=40) as p_probs:
                ktf = p_kv.tile([P, EC, S], f8, tag="ktf", name="ktf")
                # head stride padded to 66 so the DoubleRow AV pair-step
                # (2 k-chunks) is 16-byte aligned (528 = 8*66), a hard
                # dual-fp8 LdWeights ISA restriction
                vf = p_kv.tile([P, NKC, H, HD + 2], f8, tag="vf", name="vf")
                nc.vector.memset(vf[:, :, :, HD:HD + 1], 1.0)
                for pan in range(S // 512):
                    xp8 = p_xp.tile([P, EC, 512], f8, tag="xp", name="xp8")
                    for s in range(EC):
                        nc.sync.dma_start(xp8[:, s, :],
                                          xfT8[P * s:P * (s + 1), 512 * pan:512 * (pan + 1)])
                    nm, r_b = ln_stats8(xp8, 0, 512)
                    rBs = p_tmp.tile([P, 512], b16, tag="rBs", name="rBs")
                    nc.gpsimd.partition_broadcast(rBs[:], r_b[:])
                    # r as per-token COLUMNS for the V store: 4 PE transposes
                    # ([1,128] -> [128,1]) through a psum bank, one copy out.
                    # bf16 PSUM writes must be 4-byte aligned -> pad each
                    # column to a 4-byte slot ([P, 4, 2] with lane 0 used).
                    rps = p_ps.tile([P, 4, 2], b16, tag="po0", name="rps")
                    for c in range(4):
                        nc.tensor.transpose(rps[:, c, 0:1],
                                            r_b[0:1, c * P:(c + 1) * P],
                                            ones[0:1, 0:1])
                    rcol = p_sm.tile([P, 4], f32, tag="rcol", name="rcol")
                    nc.vector.tensor_copy(rcol[:], rps[:, :, 0])
                    # K^T columns for this panel (folded mean, store on DVE:
                    # ACT is the kernel-wide bottleneck with softmax exp)
                    for oc in range(EC):
                        ps = ps_tile()
                        fold_proj(ps, E + oc * P, xp8, 0, 512, nm)
                        with nc.allow_low_precision(reason="fp8 K store"):
                            nc.vector.tensor_mul(
                                ktf[:, oc, 512 * pan:512 * (pan + 1)],
                                ps[:], rBs[:])
                            if use_bias:
                                nc.vector.tensor_scalar(
                                    ktf[:, oc, 512 * pan:512 * (pan + 1)],
                                    ktf[:, oc, 512 * pan:512 * (pan + 1)],
                                    bslice(0, "qkv", EC + oc), None, OP.add)
                    # V (normal layout, rank-1 mean fold via 1-row stationary)
                    for tck in range(4):
                        ps = ps_tile()
                        for j in range(EC // 2):
                            nc.tensor.matmul(
                                ps[:],
                                xp8[:, 2 * j:2 * j + 2, tck * P:(tck + 1) * P],
                                wqkv0[:, 2 * j:2 * j + 2, 2 * E:3 * E],
                                start=(j == 0), stop=False,
                                perf_mode=PM.DoubleRow)
                        nc.tensor.matmul(ps[:], nm[0:1, tck * P:(tck + 1) * P],
                                         c8[0:1, 2 * E:3 * E],
                                         start=False, stop=True,
                                         skip_group_check=True)
                        kc = pan * 4 + tck
                        if use_bias:
                            add_vbias(0, ps)
                        # V store on ACT (Copy with per-partition r scale):
                        # the kv phase is DVE-span-bound, ACT has room here
                        act(vf[:, kc, :, 0:HD],
                            ps[:].rearrange("p (h d) -> p h d", h=H),
                            AF.Copy, scale=rcol[:, tck:tck + 1])
                mark("b0.attn")
                # attention with out-proj + MLP interleaved per query tile:
                # the exp-bound attention phase hides the PE-heavy MLP
                x1 = [p_x.tile([P, T], b16, tag=f"x{s}", name=f"x1_{s}")
                      for s in range(EC)]
                def post_out0(oc, o, n, ps):
                    if use_bias:
                        nc.vector.tensor_scalar(ps[:, :n], ps[:, :n],
                                                bslice(0, "out", oc), None, OP.add)
                    with nc.allow_low_precision(reason="bf16 residual"):
                        nc.vector.scalar_tensor_tensor(
                            x1[oc][:, o:o + n], ps[:, :n], 1.0 / BETA,
                            xs[oc][:, o:o + n], OP.mult, OP.add)
                wf1, wf2 = mlp_weights(0)
                h2 = p_h.tile([P, EC, T], f8, tag="h", name="h2_0")
                x1m = xs  # overwrite in place: xs cols die at out-proj (WAR)
                QTS = ((0, 512), (512, 512), (1024, 128))
                done_nt = [0]

                def qtile_out(q0, qn_):
                    # out-proj as soon as the qtile's AV chains are done
                    gemm8(wout, 0, EC, ot0, T, post_out0, chunks=[(q0, qn_)])

                def qtile_tail(q0, qn_):
                    # LN2 + MLP for a finished qtile (deferred one extra unit)
                    while (done_nt[0] + 1) * NT <= q0 + qn_:
                        ln_chunk(x1, h2, done_nt[0] * NT, NT, True)
                        mlp_nt(0, x1, x1m, h2, wf1, wf2, done_nt[0],
                               use_mm=True)
                        done_nt[0] += 1

                # software-pipelined over (qtile, head) units: scores+exps of
                # unit N overlap the AV chain of unit N-1 on PE, so the ACT
                # exp stream never stalls behind an AV/outproj/MLP stretch
                pend = None      # (h, q0, qn, prs, qtile_done)
                tail_todo = None  # deferred one extra unit so the PE MLP
                                  # stretch hides behind 2 heads of exp work
                for qi in range(len(QTS)):
                    q0, qn_ = QTS[qi]
                    for h_ in range(H):
                        prs = attn_scores_exps(ktf, qt_t, h_, q0, qn_)
                        if pend is not None:
                            ph, pq0, pqn, pprs, pdone = pend
                            attn_avs(vf, ot_strips, ph, pq0, pqn, pprs)
                            if tail_todo is not None:
                                qtile_tail(*tail_todo)
                                tail_todo = None
                            if pdone:
                                qtile_out(pq0, pqn)
                                tail_todo = (pq0, pqn)
                        pend = (h_, q0, qn_, prs, h_ == H - 1)
                ph, pq0, pqn, pprs, _ = pend
                attn_avs(vf, ot_strips, ph, pq0, pqn, pprs)
                if tail_todo is not None:
                    qtile_tail(*tail_todo)
                qtile_out(pq0, pqn)
                qtile_tail(pq0, pqn)
                x1 = x1m

            # ---- pools for the banded phases (opened after kvfull frees,
            # closed at end of emission so repeat>1 can reopen) ----
            _lstack = contextlib.ExitStack()
            lpool = lambda name, bufs, **kw: _lstack.enter_context(
                tc.tile_pool(name=name, bufs=bufs, **kw))
            p_kt = lpool("kt", 1)        # KT (banded) [P, EC, T] fp8
            p_v = lpool("v", 1)          # V_ext [P, NTC, H, HD+1] bf16
            p_prb = lpool("prb", 4)      # banded pre-mask probs bf16
            p_prm = lpool("prm", 10)     # banded masked probs bf16 (per head)

            # ---------- MLP (ln2 + fc1 + gelu + fc2 + residual), bf16 ----------
            def mlp(i, x_strips):
                h2 = layernorm(x_strips, T, "h", True)
                wf1, wf2 = mlp_weights(i)
                x_new = [p_x.tile([P, T], b16, tag=f"x{s}", name=f"xm{i}_{s}")
                         for s in range(EC)]
                for nt in range(NQT):
                    mlp_nt(i, x_strips, x_new, h2, wf1, wf2, nt)
                return x_new

            # ==================================================================
            # BLOCKS 1, 2: banded attention
            # ==================================================================
            mark("banded")
            x_cur = x1
            for i in (1, 2):
                lower = (i == 1)
                mark(f"b{i}.ln1qkv")
                wqkv = p_wqkv.tile([P, EC, 3 * E], f8, tag="wqkv", name=f"wqkv{i}")
                for ec in range(EC):
                    nc.sync.dma_start(wqkv[:, ec, :], wq[i][ec])
                wout = p_wout.tile([P, EC, E], f8, tag="wout", name=f"wout{i}")
                for ec in range(EC):
                    nc.sync.dma_start(wout[:, ec, :], wo[i][ec])
                h1 = layernorm(x_cur, T, "h", True)
                qt_t = p_qt.tile([P, EC, T], f8, tag="qt", name=f"qt{i}")
                kt_t = p_kt.tile([P, EC, T], f8, tag="kt", name=f"kt{i}")
                v_t = p_v.tile([P, NTC, H, HD + 1], b16, tag="v", name=f"v{i}")
                nc.vector.memset(v_t[:, :, :, HD:HD + 1], 1.0)
                otb = p_ot.tile([P, EC, T], f8, tag="ot", name=f"ot{i}")
                ot_strips = [otb[:, s, :] for s in range(EC)]
                mark(f"b{i}.attn")
                WINS = ([(0, 15), (0, 143), (128, 143), (256, 128)] if lower
                        else [(0, 143), (113, 143), (241, 143), (369, 15)])
                _gq = [0]

                def qkv_ps():
                    # only mm0/mm1 here: po/sc banks stay free for the
                    # interleaved attention chains
                    t = ("mm0", "mm1")[_gq[0] % 2]
                    _gq[0] += 1
                    return p_ps.tile([P, 512], f32, tag=t, name="gq")

                def emit_attn_qt(qt):
                    # two-phase across ALL heads. Each head's <=4 band windows
                    # are PACKED into one PSUM bank (sum <=444 cols): one
                    # score matmul per window, but only ONE exp and ONE
                    # mask-mul per head (packed mask built host-side).
                    qt0 = qt * NT
                    units = []          # (kc, qo, qw, packed off)
                    off = 0
                    for c in range(4):
                        kc = 3 * qt + (c - 1 if lower else c)
                        if 0 <= kc < NTC:
                            qo, qw = WINS[c]
                            units.append((kc, qo, qw, off))
                            off += qw
                    tot = off
                    mp = masks["mp_%s%d" % ("lo" if lower else "up",
                                            len(units))]
                    prm_of = {}
                    sps_cur = [None]
                    for h_ in range(H):
                        hp, hh = h_ // 2, h_ % 2
                        bank = h_ % 2
                        if bank == 0:
                            sps_cur[0] = sc_tile()
                        sps = sps_cur[0]
                        for (kc, qo, qw, uo) in units:
                            nc.tensor.matmul(
                                sps[:, bank, uo:uo + qw],
                                kt_t[HD * hh:HD * (hh + 1), hp,
                                     kc * P:(kc + 1) * P],
                                qt_t[HD * hh:HD * (hh + 1), hp,
                                     qt0 + qo:qt0 + qo + qw],
                                start=True, stop=True)
                        pr = p_prb.tile([P, 512], b16, tag="pr", name="prb")
                        act(pr[:, :tot], sps[:, bank, :tot], AF.Exp,
                            scale=0.125 / (GQ * GK))
                        prm = p_prm.tile([P, 512], b16, tag="prm", name="prm")
                        nc.vector.tensor_mul(prm[:, :tot], pr[:, :tot],
                                             mp[:, :tot])
                        prm_of[h_] = prm
                    for h_ in range(H):
                        hp, hh = h_ // 2, h_ % 2
                        po = po_tile()
                        covered = []
                        for (kc, qo, qw, uo) in units:
                            pr = prm_of[h_]
                            parts = []
                            pos = qo
                            for (clo, chi) in covered + [(qo + qw, qo + qw)]:
                                if pos >= qo + qw:
                                    break
                                if chi <= pos:
                                    continue
                                if clo > pos:
                                    parts.append((pos, min(clo, qo + qw), True))
                                if clo < qo + qw:
                                    lo = max(clo, pos)
                                    hi = min(chi, qo + qw)
                                    if lo < hi:
                                        parts.append((lo, hi, False))
                                pos = max(pos, chi)
                            for (lo, hi, is_new) in parts:
                                nc.tensor.matmul(
                                    po[:, lo:hi], v_t[:, kc, h_, :],
                                    pr[:, uo + lo - qo:uo + hi - qo],
                                    start=is_new, stop=False,
                                    skip_group_check=True)
                            covered.append((qo, qo + qw))
                            covered = sorted(covered)
                            merged = []
                            for (lo, hi) in covered:
                                if merged and lo <= merged[-1][1]:
                                    merged[-1] = (merged[-1][0],
                                                  max(hi, merged[-1][1]))
                                else:
                                    merged.append((lo, hi))
                            covered = merged
                        _finish_attn(po, qt0, NT, hp, hh, ot_strips,
                                     banded=True)

                # emit qkv per token tile, with each attention query tile
                # interleaved as soon as its K/V columns exist (in-order
                # engine streams otherwise serialize attention behind the
                # whole projection)
                def emit_qkv_nt(nt):
                    o0 = nt * NT
                    for oc in range(EC):
                        ps = qkv_ps()
                        for j in range(EC // 2):
                            nc.tensor.matmul(
                                ps[:, :NT],
                                wqkv[:, 2 * j:2 * j + 2, E + oc * P:E + (oc + 1) * P],
                                h1[:, 2 * j:2 * j + 2, o0:o0 + NT],
                                start=(j == 0), stop=(j == EC // 2 - 1),
                                perf_mode=PM.DoubleRow)
                        if use_bias:
                            psum_scaled(kt_t[:, oc, o0:o0 + NT], ps[:, :NT],
                                        GK / (ALPHA * BETA), i, "qkv", EC + oc)
                        else:
                            # scalar-scale stores go to ACT: the banded
                            # region is DVE-bound (ACT 40% vs DVE 64%)
                            act(kt_t[:, oc, o0:o0 + NT], ps[:, :NT],
                                AF.Copy, scale=GK / (ALPHA * BETA))
                        ps = qkv_ps()
                        for j in range(EC // 2):
                            nc.tensor.matmul(
                                ps[:, :NT],
                                wqkv[:, 2 * j:2 * j + 2, oc * P:(oc + 1) * P],
                                h1[:, 2 * j:2 * j + 2, o0:o0 + NT],
                                start=(j == 0), stop=(j == EC // 2 - 1),
                                perf_mode=PM.DoubleRow)
                        if use_bias:
                            psum_scaled(qt_t[:, oc, o0:o0 + NT], ps[:, :NT],
                                        GQ / (ALPHA * BETA), i, "qkv", oc)
                        else:
                            act(qt_t[:, oc, o0:o0 + NT], ps[:, :NT],
                                AF.Copy, scale=GQ / (ALPHA * BETA))
                    for tck in range(3 * nt, 3 * nt + 3):
                        ps = qkv_ps()
                        for j in range(EC // 2):
                            nc.tensor.matmul(
                                ps[:],
                                h1[:, 2 * j:2 * j + 2, tck * P:(tck + 1) * P],
                                wqkv[:, 2 * j:2 * j + 2, 2 * E:3 * E],
                                start=(j == 0), stop=(j == EC // 2 - 1),
                                perf_mode=PM.DoubleRow)
                        if use_bias:
                            add_vbias(i, ps)
                            with nc.allow_low_precision(reason="bf16 V store"):
                                nc.vector.tensor_scalar(
                                    v_t[:, tck, :, 0:HD],
                                    ps[:].rearrange("p (h d) -> p h d", h=H),
                                    GV / (ALPHA * BETA), None, OP.mult)
                        else:
                            # GV-scaled like the global path: the shared
                            # out-proj weights compensate by BETA/GV
                            act(v_t[:, tck, :, 0:HD],
                                ps[:].rearrange("p (h d) -> p h d", h=H),
                                AF.Copy, scale=GV / (ALPHA * BETA))
                x_new = [p_x.tile([P, T], b16, tag=f"x{s}", name=f"xa{i}_{s}")
                         for s in range(EC)]
                def post_out(oc, o, n, ps, i=i, x_new=x_new, x_cur=x_cur):
                    if use_bias:
                        nc.vector.tensor_scalar(ps[:, :n], ps[:, :n],
                                                bslice(i, "out", oc), None, OP.add)
                    with nc.allow_low_precision(reason="bf16 residual"):
                        nc.vector.scalar_tensor_tensor(
                            x_new[oc][:, o:o + n], ps[:, :n], 1.0 / BETA,
                            x_cur[oc][:, o:o + n], OP.mult, OP.add)

                for nt in range(NQT):
                    emit_qkv_nt(nt)
                    aq = nt if lower else nt - 1
                    if aq >= 0:
                        emit_attn_qt(aq)
                if not lower:
                    emit_attn_qt(NQT - 1)
                mark(f"b{i}.projmlp")
                gemm8(wout, 0, EC, otb, T, post_out)
                x_cur = mlp(i, x_new)

            mark("out")
            # output per MLP chunk so the DMA overlaps the last block's tail
            for s in range(EC):
                for nt2 in range(NQT):
                    nc.sync.dma_start(
                        yT[P * s:P * (s + 1), nt2 * NT:(nt2 + 1) * NT],
                        x_cur[s][:, nt2 * NT:(nt2 + 1) * NT])
            _lstack.close()


        for _rep in range(repeat):
            _emit_once()

        # record build-order instruction -> phase map (before scheduling)
        _PHASE_OF.clear()
        names = [ins.name for bb in nc.main_func.blocks for ins in bb.instructions]
        bounds = [n for _, n in _PHASE_MARKS]
        labels = [l for l, _ in _PHASE_MARKS]
        import bisect as _bis
        for idx, nm in enumerate(names):
            j = _bis.bisect_right(bounds, idx) - 1
            _PHASE_OF[nm] = labels[j] if j >= 0 else "pre"

    nc.compile()
    return nc


# ------------------------------------------------------------------
# cached executor (compile once, run many)
# ------------------------------------------------------------------
class _Exec:
    def __init__(self, use_bias: bool):
        import jax
        import concourse.mybir as mybir
        from concourse import bass2jax
        from concourse.bass2jax import install_neuronx_cc_hook, _bass_exec_p
        from jax.sharding import Mesh, PartitionSpec
        from jax.experimental.shard_map import shard_map

        install_neuronx_cc_hook()
        nc = _build_nc(use_bias)
        self.nc = nc

        part_name = (nc.partition_id_tensor.name
                     if nc.partition_id_tensor is not None else None)
        in_names, out_names, out_avals = [], [], []
        self.zero_shapes = []
        for alloc in nc.m.functions[0].allocations:
            if not isinstance(alloc, mybir.MemoryLocationSet):
                continue
            name = alloc.memorylocations[0].name
            if alloc.kind == "ExternalInput":
                if name != part_name:
                    in_names.append(name)
            elif alloc.kind == "ExternalOutput":
                out_names.append(name)
                shape = tuple(alloc.tensor_shape)
                dtype = mybir.dt.np(alloc.dtype)
                out_avals.append(jax.core.ShapedArray(shape, dtype))
                self.zero_shapes.append((shape, dtype))
        n_params = len(in_names)
        all_in = in_names + out_names
        if part_name is not None:
            all_in = all_in + [part_name]
        self.in_names = in_names
        self.out_names = out_names
        n_outs = len(out_names)

        def _body(*args):
            operands = list(args)
            if part_name is not None:
                operands.append(bass2jax.partition_id_tensor())
            outs = _bass_exec_p.bind(
                *operands,
                out_avals=tuple(out_avals),
                in_names=tuple(all_in),
                out_names=tuple(out_names),
                lowering_input_output_aliases=(),
                sim_require_finite=True,
                sim_require_nnan=True,
                nc=nc,
            )
            return tuple(outs)
        self._body = _body

        devices = jax.devices()[:N_CORES]
        mesh = Mesh(np.asarray(devices), ("core",))
        in_specs = (PartitionSpec("core"),) * (n_params + n_outs)
        out_specs = (PartitionSpec("core"),) * n_outs
        donate = tuple(range(n_params, n_params + n_outs))
        self.fn = jax.jit(
            shard_map(_body, mesh=mesh, in_specs=in_specs,
                      out_specs=out_specs, check_rep=False),
            donate_argnums=donate, keep_unused=True)
        self.out_avals = out_avals

    def bench(self, in_maps, iters=10):
        """device-resident-input timing: returns per-iteration seconds"""
        import time
        import jax
        from jax.sharding import Mesh, PartitionSpec, NamedSharding
        if not hasattr(self, "_bench_fn"):
            from jax.experimental.shard_map import shard_map
            devices = jax.devices()[:N_CORES]
            mesh = Mesh(np.asarray(devices), ("core",))
            n_in = len(self.in_names) + len(self.zero_shapes)
            self._bench_fn = jax.jit(
                shard_map(self._body, mesh=mesh,
                          in_specs=(PartitionSpec("core"),) * n_in,
                          out_specs=(PartitionSpec("core"),) * len(self.out_names),
                          check_rep=False),
                keep_unused=True)
            self._bench_sharding = NamedSharding(mesh, PartitionSpec("core"))
        concat_in = [
            np.concatenate([np.asarray(in_maps[c][n]) for c in range(N_CORES)], axis=0)
            for n in self.in_names
        ] + [np.zeros((N_CORES * s[0], *s[1:]), d) for (s, d) in self.zero_shapes]
        import jax
        dev_in = [jax.device_put(a, self._bench_sharding) for a in concat_in]
        out = jax.block_until_ready(self._bench_fn(*dev_in))  # warm/compile
        t0 = time.time()
        for _ in range(iters):
            out = self._bench_fn(*dev_in)
        jax.block_until_ready(out)
        return (time.time() - t0) / iters

    def run(self, in_maps):
        """in_maps: list of 8 dicts name->np.ndarray. returns list of dicts"""
        concat_in = [
            np.concatenate([np.asarray(in_maps[c][n]) for c in range(N_CORES)], axis=0)
            for n in self.in_names
        ]
        concat_zeros = [np.zeros((N_CORES * s[0], *s[1:]), d)
                        for (s, d) in self.zero_shapes]
        outs = self.fn(*concat_in, *concat_zeros)
        import jax
        outs = jax.block_until_ready(outs)
        res = []
        for c in range(N_CORES):
            d = {}
            for idx, n in enumerate(self.out_names):
                shp = self.out_avals[idx].shape
                d[n] = np.asarray(outs[idx]).reshape(N_CORES, *shp)[c]
            res.append(d)
        return res


def _get_exec(use_bias: bool) -> "_Exec":
    key = bool(use_bias)
    if key not in _EXEC_CACHE:
        _EXEC_CACHE[key] = _Exec(key)
    return _EXEC_CACHE[key]


# ------------------------------------------------------------------
# host-side input prep
# ------------------------------------------------------------------
def _band_masks():
    ki = np.arange(P)[:, None]
    qi = np.arange(NT)[None, :]
    base = {}
    for c in range(4):
        # lower band, k-chunk kc = 3t + (c-1):  0 <= (i-k) <= W-1 with
        # i-k = qi - ki + 128*(1-c)
        d = qi - ki + P * (1 - c)
        base[f"lo{c}"] = ((d >= 0) & (d <= W_BAND - 1)).astype(np.float32)
        # upper band, k-chunk kc = 3t + c:  0 <= (k-i) <= W-1 with
        # k-i = ki - qi + 128*c
        d = ki - qi + P * c
        base[f"up{c}"] = ((d >= 0) & (d <= W_BAND - 1)).astype(np.float32)
    WINS_LO = [(0, 15), (0, 143), (128, 143), (256, 128)]
    WINS_UP = [(0, 143), (113, 143), (241, 143), (369, 15)]

    def pack(band, cs, wins):
        segs = [base[f"{band}{c}"][:, qo:qo + qw]
                for c, (qo, qw) in zip(cs, [wins[c] for c in cs])]
        a = np.concatenate(segs, 1)
        out = np.zeros((P, 448), np.float32)
        out[:, :a.shape[1]] = a
        return out.astype(bf16)

    return {
        "mp_lo3": pack("lo", [1, 2, 3], WINS_LO),
        "mp_lo4": pack("lo", [0, 1, 2, 3], WINS_LO),
        "mp_up4": pack("up", [0, 1, 2, 3], WINS_UP),
        "mp_up3": pack("up", [0, 1, 2], WINS_UP),
    }


def kernel(x, ln1_w, ln1_b, ln2_w, ln2_b, qkv_w, qkv_b, out_w, out_b,
           fc1_w, fc1_b, fc2_w, fc2_b):
    x = np.asarray(x, np.float32)
    ln1_w = np.asarray(ln1_w, np.float32); ln1_b = np.asarray(ln1_b, np.float32)
    ln2_w = np.asarray(ln2_w, np.float32); ln2_b = np.asarray(ln2_b, np.float32)
    qkv_w = np.asarray(qkv_w, np.float32); qkv_b = np.asarray(qkv_b, np.float32)
    out_w = np.asarray(out_w, np.float32); out_b = np.asarray(out_b, np.float32)
    fc1_w = np.asarray(fc1_w, np.float32); fc1_b = np.asarray(fc1_b, np.float32)
    fc2_w = np.asarray(fc2_w, np.float32); fc2_b = np.asarray(fc2_b, np.float32)

    # fold LN affine into following projection weights (exact):
    #   h = z*w + b  =>  h @ Wt.T = z @ (W*w).T + b @ W.T
    wq_f, wo_f, w1_f, w2_f = [], [], [], []
    biases = []
    for i in range(3):
        qw = qkv_w[i] * ln1_w[i][None, :]
        qb = qkv_b[i] + qkv_w[i] @ ln1_b[i]
        f1 = fc1_w[i] * ln2_w[i][None, :]
        f1b = fc1_b[i] + fc1_w[i] @ ln2_b[i]
        wq_f.append(qw); w1_f.append(f1)
        wo_f.append(out_w[i]); w2_f.append(fc2_w[i])
        # packed per-feature bias tile [P, 32] (q, k, out, fc1, fc2) + v row,
        # pre-scaled to match the device-side psum compensation factors:
        # q rows x GQ (fp8 Q store), k raw, out/fc raw; v row x GV (bf16)
        bias_pack = np.zeros((P, 32), np.float32)
        bias_pack[:, 0:4] = (GQ * qb[:E]).reshape(4, P).T
        bias_pack[:, 4:8] = (GK * qb[E:2 * E]).reshape(4, P).T
        bias_pack[:, 8:12] = out_b[i].reshape(4, P).T
        # fc1 runs fp8: its psum carries ALPHA*BETA scale; fc2 stays bf16
        bias_pack[:, 12:28] = (ALPHA * BETA * f1b).reshape(16, P).T
        bias_pack[:, 28:32] = fc2_b[i].reshape(4, P).T
        biases.append((bias_pack, (ALPHA * BETA * qb[2 * E:]).reshape(1, E).astype(bf16)))
    use_bias = any(np.abs(b).max() > 0 or np.abs(np.asarray(v, np.float32)).max() > 0
                   for b, v in biases)

    ex = _get_exec(use_bias)

    masks = _band_masks()
    # weight tensors, transposed to [*, P, out] layout; qkv/out in scaled fp8
    weights = {}
    for i in range(3):
        weights[f"qkvT8_{i}"] = np.ascontiguousarray(
            wq_f[i].T.reshape(EC, P, 3 * E) * BETA).astype(e4m3)
        weights[f"outT8_{i}"] = np.ascontiguousarray(
            wo_f[i].T.reshape(EC, P, E) * (BETA / GV)).astype(e4m3)
        weights[f"fc1T{i}"] = np.ascontiguousarray(
            w1_f[i].T.reshape(EC, P, FF) * BETA).astype(e4m3)
        weights[f"fc2T{i}"] = np.ascontiguousarray(
            w2_f[i].T.reshape(FFC, P, E)).astype(bf16)
        if use_bias:
            weights[f"bias{i}"] = biases[i][0]
            weights[f"vbias{i}"] = biases[i][1]

    # block-0 folded-LN path: AX-scaled fp8 x and qkv0 weight column sums
    # (sums over the QUANTIZED fp8 weights, so the fold is self-consistent)
    weights["csum0"] = weights["qkvT8_0"].astype(np.float32).reshape(
        E, 3 * E).sum(axis=0).reshape(1, 3 * E).astype(bf16)

    in_maps = []
    for core in range(N_CORES):
        b, c = divmod(core, CHUNKS)
        e0 = EXT_STARTS[c]
        xT_ext = np.ascontiguousarray(x[b, e0:e0 + T, :].T).astype(bf16)
        xT8_ext = np.ascontiguousarray(
            x[b, e0:e0 + T, :].T * ALPHA).astype(e4m3)
        xfT8 = np.ascontiguousarray(x[b].T * ALPHA).astype(e4m3)
        m = {"xT": xT_ext, "xT8": xT8_ext, "xfT8": xfT8, **weights, **masks}
        in_maps.append(m)

    res = ex.run(in_maps)

    out = np.empty((B, S, E), np.float32)
    for core in range(N_CORES):
        b, c = divmod(core, CHUNKS)
        yT_ = res[core]["yT"]               # [E, T] bf16
        off = OWN_OFF[c]
        out[b, OWN * c:OWN * (c + 1), :] = yT_[:, off:off + OWN].T.astype(np.float32)
    return out

